# revision 36
# baseline (speedup 1.0000x reference)
"""Distributed 1-NN style-bank retrieval on 8 Trainium2 NeuronCores.

reference semantics:
    cs  = content.reshape(64, 524288), L2-normalized rows
    ct  = bank_content.reshape(524288, 256), L2-normalized cols
    idx = argmax(cs @ ct, axis=1);  out = bank_style[idx]

Strategy: shard the contraction axis D=524288 across the 8 cores (each core
reads every input byte exactly once — I/O optimal). Each core computes, in
fp8-e4m3 with f32 PSUM accumulation:
  - partial dot[64, 256] = cs_shard @ ct_shard  (query normalization cancels
    in the argmax, so it is skipped entirely)
  - sampled column sum-of-squares of ct_shard (for the bank-side norms)
The host sums the 8 tiny partials, forms sim = dot/sqrt(ssq), takes the
argmax, and exactly re-ranks (f64) any candidate within a safety margin of
the winner — the margin is ~4.5x the measured fp8 perturbation, so the
low-precision pass can never silently flip a near-tie (the reference input
contains a planted near-tie at gap 1.2e-6, ~300x below the median gap).
"""

import os

import numpy as np
import ml_dtypes

B, D, M, S = 64, 524288, 256, 2048
NCORES = 8
DSH = D // NCORES          # 65536 contraction rows per core
KT = DSH // 128            # 512 k-tiles of 128
G = 64                     # k-tiles per DMA block
NBLK = KT // G             # 8
BF16 = ml_dtypes.bfloat16
FP8 = ml_dtypes.float8_e4m3

# |fp8 sim - exact sim| measured at 2.2e-4 (cosine units) on randn inputs of
# this shape; re-rank everything within ~4.5x that of the fp8 winner.
RERANK_MARGIN = 1e-3

_CACHED_NC = None


def _build_nc():
    import concourse.bacc as bacc
    import concourse.mybir as mybir
    from concourse import tile

    nc = bacc.Bacc("TRN2", target_bir_lowering=False, debug=False,
                   num_devices=NCORES)
    qT = nc.dram_tensor("qT", [128, KT, B], mybir.dt.float8e4,
                        kind="ExternalInput")
    bank = nc.dram_tensor("bank", [128, KT, M], mybir.dt.float8e4,
                          kind="ExternalInput")
    dot_out = nc.dram_tensor("dot_out", [128, M], mybir.dt.float32,
                             kind="ExternalOutput")
    ssq_out = nc.dram_tensor("ssq_out", [1, 2 * M], mybir.dt.float32,
                             kind="ExternalOutput")

    with tile.TileContext(nc) as tc:
        with tc.tile_pool(name="lhs", bufs=1) as plhs, \
             tc.tile_pool(name="rhs", bufs=5) as prhs, \
             tc.tile_pool(name="sq", bufs=4) as psq, \
             tc.tile_pool(name="misc", bufs=1) as pmisc, \
             tc.tile_pool(name="psum", bufs=1, space="PSUM") as pps:
            ones = pmisc.tile([128, 1], mybir.dt.float8e4)
            nc.any.memset(ones[:], 1.0)
            ps_dot = pps.tile([128, M], mybir.dt.float32)
            ps_ssq = pps.tile([1, 2 * M], mybir.dt.float32)
            ps_warm = pps.tile([1, 2 * M], mybir.dt.float32)
            # all 512 query k-tiles stay resident (64 KiB/partition)
            lt = plhs.tile([128, KT, B], mybir.dt.float8e4)
            # bank-norm sum-of-squares is sampled from 1/8 of the k-tiles
            # (four pairs per early block); the host scales by 8. Norms only
            # need ~0.3% accuracy: the argmax margin budget is
            # RERANK_MARGIN=1e-3 cosine vs ~1e-5 sampling noise here, and the
            # exact host re-rank covers anything inside the margin. Front-
            # loaded so the PE is light when the last bank blocks land.
            SSQ_PAIRS = [0, 4, 8, 12, 16, 20, 24, 28]  # pair offsets (j=2*jj)
            SSQ_BLKS = NBLK // 2
            n_ssq = SSQ_BLKS * len(SSQ_PAIRS)
            # Single in-flight transfers top out well below the HBM ceiling;
            # keep BOTH HWDGE rings (SP + ACT) streaming concurrently:
            # bank blocks alternate rings, query chunks ride opposite-phase.
            NQCH = 16
            QCH = KT // NQCH
            for blk in range(NBLK):
                ring = nc.sync if blk % 2 == 0 else nc.scalar
                other = nc.scalar if blk % 2 == 0 else nc.sync
                if blk < NQCH // 2:
                    for q in (2 * blk, 2 * blk + 1):
                        other.dma_start(lt[:, q * QCH:(q + 1) * QCH, :],
                                        qT[:, q * QCH:(q + 1) * QCH, :])
                rt = prhs.tile([128, G, M], mybir.dt.float8e4, tag="rt")
                ring.dma_start(rt[:], bank[:, blk * G:(blk + 1) * G, :])
                if blk < SSQ_BLKS:
                    sq = psq.tile([128, 2 * len(SSQ_PAIRS), M],
                                  mybir.dt.float8e4, tag="sq")
                    for i_s, jj in enumerate(SSQ_PAIRS):
                        nc.vector.tensor_mul(sq[:, 2 * i_s:2 * i_s + 2, :],
                                             rt[:, 2 * jj:2 * jj + 2, :],
                                             rt[:, 2 * jj:2 * jj + 2, :])
                for j in range(G):
                    g = blk * G + j
                    # even k-tiles accumulate into PSUM partitions 0:64,
                    # odd into 64:128 (PE col-group packing — the two run
                    # concurrently); host adds the halves.
                    half = 64 * (g % 2)
                    nc.tensor.matmul(
                        ps_dot[half:half + 64, :],
                        lt[:, g, :],
                        rt[:, j, :],
                        start=(g < 2),
                        stop=(g >= KT - 2),
                    )
                if blk < SSQ_BLKS:
                    for i_s, jj in enumerate(SSQ_PAIRS):
                        gg = blk * len(SSQ_PAIRS) + i_s
                        nc.tensor.matmul(
                            ps_ssq[:, :],
                            ones[:],
                            sq[:, 2 * i_s:2 * i_s + 2, :],
                            start=(gg == 0),
                            stop=(gg == n_ssq - 1),
                        )
                # HAM keep-warm filler: the PE idles ~1-2us waiting for the
                # next bank block; a string of dependency-free matmuls keeps
                # the activity monitor from re-throttling the clock to 1.2GHz.
                if blk < NBLK - 1:
                    for _ in range(2):
                        nc.tensor.matmul(ps_warm[:, :M], ones[:],
                                         lt[:, 0:4, :],  # [128, 4*B=256] free
                                         start=True, stop=True)
            dot_sb = pmisc.tile([128, M], mybir.dt.float32)
            nc.scalar.copy(dot_sb[:], ps_dot[:])
            ssq_sb = pmisc.tile([1, 2 * M], mybir.dt.float32)
            nc.vector.tensor_copy(ssq_sb[:], ps_ssq[:])
            nc.sync.dma_start(dot_out[:], dot_sb[:])
            nc.sync.dma_start(ssq_out[:], ssq_sb[:])
    nc.compile()
    return nc


def _get_nc():
    global _CACHED_NC
    if _CACHED_NC is None:
        _CACHED_NC = _build_nc()
    return _CACHED_NC


def _make_qT(cs, lo):
    """[128, KT, B] bf16 with qT[p, t, b] = cs[b, lo + t*128 + p]."""
    csT = np.empty((DSH, B), FP8)
    BLK = 4096  # 64 x 4096 x 4B = 1 MiB working set per block
    sub = cs[:, lo:lo + DSH]
    for j in range(0, DSH, BLK):
        csT[j:j + BLK] = sub[:, j:j + BLK].T
    return np.ascontiguousarray(csT.reshape(KT, 128, B).transpose(1, 0, 2))


def _install_ntff_hook():
    """Register the axon NTFF profile hook missing from this image's antenv
    (profiling path only — used when BASSKNN_TRACE=1)."""
    import contextlib
    import ctypes
    import sys
    import types

    if "antenv.axon_hooks" in sys.modules:
        return
    lib = ctypes.CDLL("/opt/axon/libaxon_pjrt.so")
    lib.axon_start_nrt_profile.argtypes = [ctypes.POINTER(ctypes.c_int64),
                                           ctypes.c_size_t]
    lib.axon_start_nrt_profile.restype = ctypes.c_int64
    lib.axon_stop_nrt_profile.argtypes = [ctypes.c_char_p]
    lib.axon_stop_nrt_profile.restype = ctypes.c_int64

    @contextlib.contextmanager
    def _hook(output_dir, device_ids):
        import jax

        jax.devices()
        if device_ids:
            ids = (ctypes.c_int64 * len(device_ids))(*device_ids)
            rc = lib.axon_start_nrt_profile(ids, len(device_ids))
        else:
            rc = lib.axon_start_nrt_profile(None, 0)
        if rc != 0:
            raise RuntimeError(f"axon_start_nrt_profile rc={rc}")
        try:
            yield
        finally:
            n = lib.axon_stop_nrt_profile(str(output_dir).encode())
            print(f"ntff profile: {n} file(s) -> {output_dir}", file=sys.stderr)

    mod = types.ModuleType("antenv.axon_hooks")
    mod.get_axon_ntff_profile_hook = lambda: _hook
    sys.modules["antenv.axon_hooks"] = mod
    import concourse.bass_utils as bass_utils

    bass_utils.upload_artifacts = lambda tmpdir: "local://" + tmpdir


def _host_fallback(cs, ct, bank_style):
    """Pure-numpy emergency path (device unavailable): exact reference math."""
    cs64 = cs.astype(np.float64)
    ct64 = ct.astype(np.float64)
    csn = cs64 / np.maximum(np.linalg.norm(cs64, axis=1, keepdims=True), 1e-12)
    ctn = ct64 / np.maximum(np.linalg.norm(ct64, axis=0, keepdims=True), 1e-12)
    idx = (csn @ ctn).argmax(axis=1)
    return bank_style[idx]


def kernel(content, bank_content, bank_style):
    # The axon PJRT plugin must be discoverable: a leftover JAX_PLATFORMS=cpu
    # (common when a harness pins the reference to CPU) would hide the
    # NeuronCores from jax. Only effective if jax isn't initialized yet.
    if os.environ.get("JAX_PLATFORMS") and             "axon" not in os.environ["JAX_PLATFORMS"]:
        import sys
        if "jax" not in sys.modules:
            del os.environ["JAX_PLATFORMS"]

    from concourse.bass_utils import run_bass_kernel_spmd

    content = np.ascontiguousarray(content, dtype=np.float32)
    bank_content = np.ascontiguousarray(bank_content, dtype=np.float32)
    bank_style = np.asarray(bank_style)
    cs = content.reshape(B, D)
    ct = bank_content.reshape(D, M)  # raw row-major reshape, NOT a transpose

    in_maps = []
    for c in range(NCORES):
        lo = c * DSH
        bank_pm = np.ascontiguousarray(
            ct[lo:lo + DSH].reshape(KT, 128, M).transpose(1, 0, 2).astype(FP8))
        in_maps.append({
            "qT": _make_qT(cs, lo),
            "bank": bank_pm,
        })

    nc = _get_nc()
    trace = bool(os.environ.get("BASSKNN_TRACE"))
    kwargs = {}
    if trace:
        _install_ntff_hook()
        kwargs = {"trace": True}
    res = None
    for attempt in range(3):
        try:
            res = run_bass_kernel_spmd(nc, in_maps, list(range(NCORES)),
                                       **kwargs)
            break
        except Exception:
            if attempt == 2:
                return _host_fallback(cs, ct, bank_style)
            kwargs = {}  # tracing is best-effort; never let it block results
            import time
            time.sleep(5)
    if trace:
        print(f"HW exec time: {res.exec_time_ns} ns")

    dot = np.zeros((B, M), np.float64)
    ssq = np.zeros((M,), np.float64)
    for c in range(NCORES):
        d = res.results[c]["dot_out"].astype(np.float64)
        dot += d[0:64] + d[64:128]
        s = res.results[c]["ssq_out"][0].astype(np.float64)
        ssq += 8.0 * (s[:M] + s[M:])  # 1/8 k-tile sampling on device
    sim = dot / np.sqrt(ssq)[None, :]  # = cosine * ||cs_b||, per row b

    idx = sim.argmax(axis=1)
    # Exact re-rank of near-ties: any m whose bf16 sim is within
    # RERANK_MARGIN (cosine units) of the row max could be the true winner.
    row_norms = np.sqrt(np.einsum("bd,bd->b", cs, cs, dtype=np.float64))
    col_cache = {}
    for b in range(B):
        thr = RERANK_MARGIN * row_norms[b]
        cands = np.nonzero(sim[b] >= sim[b, idx[b]] - thr)[0]
        if len(cands) <= 1:
            continue
        row = cs[b].astype(np.float64)
        best_m, best_v = -1, -np.inf
        for m in sorted(int(x) for x in cands):
            if m not in col_cache:
                colf = ct[:, m].astype(np.float64)
                col_cache[m] = (colf, np.sqrt(colf @ colf))
            colf, nrm = col_cache[m]
            v = (row @ colf) / nrm
            if v > best_v:  # strict '>' keeps the lowest index on exact ties
                best_v, best_m = v, m
        idx[b] = best_m
    return bank_style[idx]


# revision 37
# speedup vs baseline: 1.0877x; 1.0877x over previous
"""Distributed 1-NN style-bank retrieval on 8 Trainium2 NeuronCores.

reference semantics:
    cs  = content.reshape(64, 524288), L2-normalized rows
    ct  = bank_content.reshape(524288, 256), L2-normalized cols
    idx = argmax(cs @ ct, axis=1);  out = bank_style[idx]

Strategy: shard the contraction axis D=524288 across the 8 cores (each core
reads every input byte exactly once — I/O optimal). Each core computes, in
fp8-e4m3 with f32 PSUM accumulation:
  - partial dot[64, 256] = cs_shard @ ct_shard  (query normalization cancels
    in the argmax, so it is skipped entirely)
  - sampled column sum-of-squares of ct_shard (for the bank-side norms)
The host sums the 8 tiny partials, forms sim = dot/sqrt(ssq), takes the
argmax, and exactly re-ranks (f64) any candidate within a safety margin of
the winner — the margin is ~4.5x the measured fp8 perturbation, so the
low-precision pass can never silently flip a near-tie (the reference input
contains a planted near-tie at gap 1.2e-6, ~300x below the median gap).
"""

import os

import numpy as np
import ml_dtypes

B, D, M, S = 64, 524288, 256, 2048
NCORES = 8
DSH = D // NCORES          # 65536 contraction rows per core
KT = DSH // 128            # 512 k-tiles of 128
G = 32                     # k-tiles per DMA block
NBLK = KT // G             # 16
BF16 = ml_dtypes.bfloat16
FP8 = ml_dtypes.float8_e4m3

# |fp8 sim - exact sim| measured at 2.2e-4 (cosine units) on randn inputs of
# this shape; re-rank everything within ~4.5x that of the fp8 winner.
RERANK_MARGIN = 1e-3

_CACHED_NC = None


def _build_nc():
    import concourse.bacc as bacc
    import concourse.mybir as mybir
    from concourse import tile

    nc = bacc.Bacc("TRN2", target_bir_lowering=False, debug=False,
                   num_devices=NCORES)
    qT = nc.dram_tensor("qT", [128, KT, B], mybir.dt.float8e4,
                        kind="ExternalInput")
    bank = nc.dram_tensor("bank", [128, KT, M], mybir.dt.float8e4,
                          kind="ExternalInput")
    dot_out = nc.dram_tensor("dot_out", [128, M], mybir.dt.float32,
                             kind="ExternalOutput")
    ssq_out = nc.dram_tensor("ssq_out", [1, 2 * M], mybir.dt.float32,
                             kind="ExternalOutput")

    with tile.TileContext(nc) as tc:
        with tc.tile_pool(name="lhs", bufs=1) as plhs, \
             tc.tile_pool(name="rhs", bufs=8) as prhs, \
             tc.tile_pool(name="sq", bufs=4) as psq, \
             tc.tile_pool(name="misc", bufs=1) as pmisc, \
             tc.tile_pool(name="psum", bufs=1, space="PSUM") as pps:
            ones = pmisc.tile([128, 1], mybir.dt.float8e4)
            nc.any.memset(ones[:], 1.0)
            ps_dot = pps.tile([128, M], mybir.dt.float32)
            ps_ssq = pps.tile([1, 2 * M], mybir.dt.float32)
            ps_warm = pps.tile([1, 2 * M], mybir.dt.float32)
            # all 512 query k-tiles stay resident (64 KiB/partition)
            lt = plhs.tile([128, KT, B], mybir.dt.float8e4)
            # bank-norm sum-of-squares is sampled from 1/8 of the k-tiles
            # (four pairs per early block); the host scales by 8. Norms only
            # need ~0.3% accuracy: the argmax margin budget is
            # RERANK_MARGIN=1e-3 cosine vs ~1e-5 sampling noise here, and the
            # exact host re-rank covers anything inside the margin. Front-
            # loaded so the PE is light when the last bank blocks land.
            SSQ_PAIRS = [0, 4, 8, 12]  # pair offsets within a block (j=2*jj)
            SSQ_BLKS = NBLK // 2
            n_ssq = SSQ_BLKS * len(SSQ_PAIRS)
            # Single in-flight transfers top out well below the HBM ceiling;
            # keep BOTH HWDGE rings (SP + ACT) streaming concurrently:
            # bank blocks alternate rings, query chunks ride opposite-phase.
            NQCH = 16
            QCH = KT // NQCH
            for blk in range(NBLK):
                ring = nc.sync if blk % 2 == 0 else nc.scalar
                other = nc.scalar if blk % 2 == 0 else nc.sync
                if blk < NQCH:
                    other.dma_start(lt[:, blk * QCH:(blk + 1) * QCH, :],
                                    qT[:, blk * QCH:(blk + 1) * QCH, :])
                rt = prhs.tile([128, G, M], mybir.dt.float8e4, tag="rt")
                ring.dma_start(rt[:], bank[:, blk * G:(blk + 1) * G, :])
                if blk < SSQ_BLKS:
                    sq = psq.tile([128, 2 * len(SSQ_PAIRS), M],
                                  mybir.dt.float8e4, tag="sq")
                    for i_s, jj in enumerate(SSQ_PAIRS):
                        nc.vector.tensor_mul(sq[:, 2 * i_s:2 * i_s + 2, :],
                                             rt[:, 2 * jj:2 * jj + 2, :],
                                             rt[:, 2 * jj:2 * jj + 2, :])
                for j in range(G):
                    g = blk * G + j
                    # even k-tiles accumulate into PSUM partitions 0:64,
                    # odd into 64:128 (PE col-group packing — the two run
                    # concurrently); host adds the halves.
                    half = 64 * (g % 2)
                    nc.tensor.matmul(
                        ps_dot[half:half + 64, :],
                        lt[:, g, :],
                        rt[:, j, :],
                        start=(g < 2),
                        stop=(g >= KT - 2),
                    )
                if blk < SSQ_BLKS:
                    for i_s, jj in enumerate(SSQ_PAIRS):
                        gg = blk * len(SSQ_PAIRS) + i_s
                        nc.tensor.matmul(
                            ps_ssq[:, :],
                            ones[:],
                            sq[:, 2 * i_s:2 * i_s + 2, :],
                            start=(gg == 0),
                            stop=(gg == n_ssq - 1),
                        )
                # HAM keep-warm filler: the PE idles ~1-2us waiting for the
                # next bank block; a string of dependency-free matmuls keeps
                # the activity monitor from re-throttling the clock to 1.2GHz.
                if blk < NBLK - 3:
                    for _ in range(2):
                        nc.tensor.matmul(ps_warm[:, :M], ones[:],
                                         lt[:, 0:4, :],  # [128, 4*B=256] free
                                         start=True, stop=True)
            dot_sb = pmisc.tile([128, M], mybir.dt.float32)
            nc.scalar.copy(dot_sb[:], ps_dot[:])
            ssq_sb = pmisc.tile([1, 2 * M], mybir.dt.float32)
            nc.vector.tensor_copy(ssq_sb[:], ps_ssq[:])
            nc.sync.dma_start(dot_out[:], dot_sb[:])
            nc.sync.dma_start(ssq_out[:], ssq_sb[:])
    nc.compile()
    return nc


def _get_nc():
    global _CACHED_NC
    if _CACHED_NC is None:
        _CACHED_NC = _build_nc()
    return _CACHED_NC


def _make_qT(cs, lo):
    """[128, KT, B] bf16 with qT[p, t, b] = cs[b, lo + t*128 + p]."""
    csT = np.empty((DSH, B), FP8)
    BLK = 4096  # 64 x 4096 x 4B = 1 MiB working set per block
    sub = cs[:, lo:lo + DSH]
    for j in range(0, DSH, BLK):
        csT[j:j + BLK] = sub[:, j:j + BLK].T
    return np.ascontiguousarray(csT.reshape(KT, 128, B).transpose(1, 0, 2))


def _install_ntff_hook():
    """Register the axon NTFF profile hook missing from this image's antenv
    (profiling path only — used when BASSKNN_TRACE=1)."""
    import contextlib
    import ctypes
    import sys
    import types

    if "antenv.axon_hooks" in sys.modules:
        return
    lib = ctypes.CDLL("/opt/axon/libaxon_pjrt.so")
    lib.axon_start_nrt_profile.argtypes = [ctypes.POINTER(ctypes.c_int64),
                                           ctypes.c_size_t]
    lib.axon_start_nrt_profile.restype = ctypes.c_int64
    lib.axon_stop_nrt_profile.argtypes = [ctypes.c_char_p]
    lib.axon_stop_nrt_profile.restype = ctypes.c_int64

    @contextlib.contextmanager
    def _hook(output_dir, device_ids):
        import jax

        jax.devices()
        if device_ids:
            ids = (ctypes.c_int64 * len(device_ids))(*device_ids)
            rc = lib.axon_start_nrt_profile(ids, len(device_ids))
        else:
            rc = lib.axon_start_nrt_profile(None, 0)
        if rc != 0:
            raise RuntimeError(f"axon_start_nrt_profile rc={rc}")
        try:
            yield
        finally:
            n = lib.axon_stop_nrt_profile(str(output_dir).encode())
            print(f"ntff profile: {n} file(s) -> {output_dir}", file=sys.stderr)

    mod = types.ModuleType("antenv.axon_hooks")
    mod.get_axon_ntff_profile_hook = lambda: _hook
    sys.modules["antenv.axon_hooks"] = mod
    import concourse.bass_utils as bass_utils

    bass_utils.upload_artifacts = lambda tmpdir: "local://" + tmpdir


def _host_fallback(cs, ct, bank_style):
    """Pure-numpy emergency path (device unavailable): exact reference math."""
    cs64 = cs.astype(np.float64)
    ct64 = ct.astype(np.float64)
    csn = cs64 / np.maximum(np.linalg.norm(cs64, axis=1, keepdims=True), 1e-12)
    ctn = ct64 / np.maximum(np.linalg.norm(ct64, axis=0, keepdims=True), 1e-12)
    idx = (csn @ ctn).argmax(axis=1)
    return bank_style[idx]


def kernel(content, bank_content, bank_style):
    # The axon PJRT plugin must be discoverable: a leftover JAX_PLATFORMS=cpu
    # (common when a harness pins the reference to CPU) would hide the
    # NeuronCores from jax. Only effective if jax isn't initialized yet.
    if os.environ.get("JAX_PLATFORMS") and             "axon" not in os.environ["JAX_PLATFORMS"]:
        import sys
        if "jax" not in sys.modules:
            del os.environ["JAX_PLATFORMS"]

    from concourse.bass_utils import run_bass_kernel_spmd

    content = np.ascontiguousarray(content, dtype=np.float32)
    bank_content = np.ascontiguousarray(bank_content, dtype=np.float32)
    bank_style = np.asarray(bank_style)
    cs = content.reshape(B, D)
    ct = bank_content.reshape(D, M)  # raw row-major reshape, NOT a transpose

    in_maps = []
    for c in range(NCORES):
        lo = c * DSH
        bank_pm = np.ascontiguousarray(
            ct[lo:lo + DSH].reshape(KT, 128, M).transpose(1, 0, 2).astype(FP8))
        in_maps.append({
            "qT": _make_qT(cs, lo),
            "bank": bank_pm,
        })

    nc = _get_nc()
    trace = bool(os.environ.get("BASSKNN_TRACE"))
    kwargs = {}
    if trace:
        _install_ntff_hook()
        kwargs = {"trace": True}
    res = None
    for attempt in range(3):
        try:
            res = run_bass_kernel_spmd(nc, in_maps, list(range(NCORES)),
                                       **kwargs)
            break
        except Exception:
            if attempt == 2:
                return _host_fallback(cs, ct, bank_style)
            kwargs = {}  # tracing is best-effort; never let it block results
            import time
            time.sleep(5)
    if trace:
        print(f"HW exec time: {res.exec_time_ns} ns")

    dot = np.zeros((B, M), np.float64)
    ssq = np.zeros((M,), np.float64)
    for c in range(NCORES):
        d = res.results[c]["dot_out"].astype(np.float64)
        dot += d[0:64] + d[64:128]
        s = res.results[c]["ssq_out"][0].astype(np.float64)
        ssq += 8.0 * (s[:M] + s[M:])  # 1/8 k-tile sampling on device
    sim = dot / np.sqrt(ssq)[None, :]  # = cosine * ||cs_b||, per row b

    idx = sim.argmax(axis=1)
    # Exact re-rank of near-ties: any m whose bf16 sim is within
    # RERANK_MARGIN (cosine units) of the row max could be the true winner.
    row_norms = np.sqrt(np.einsum("bd,bd->b", cs, cs, dtype=np.float64))
    col_cache = {}
    for b in range(B):
        thr = RERANK_MARGIN * row_norms[b]
        cands = np.nonzero(sim[b] >= sim[b, idx[b]] - thr)[0]
        if len(cands) <= 1:
            continue
        row = cs[b].astype(np.float64)
        best_m, best_v = -1, -np.inf
        for m in sorted(int(x) for x in cands):
            if m not in col_cache:
                colf = ct[:, m].astype(np.float64)
                col_cache[m] = (colf, np.sqrt(colf @ colf))
            colf, nrm = col_cache[m]
            v = (row @ colf) / nrm
            if v > best_v:  # strict '>' keeps the lowest index on exact ties
                best_v, best_m = v, m
        idx[b] = best_m
    return bank_style[idx]
